# revision 14
# baseline (speedup 1.0000x reference)
"""Trainium2 Bass kernel: fused cross-head attention block (data parallel, 8 cores).

Problem (hardcoded shapes): x_c, x_t [8, 256, 128, 128] f32; Wq/Wk/Wv/Wo
[256, 256]; biases [256]; out [8, 256, 128, 128] f32.

Math per sample (C=256, nh=8, hd=32, N=H*W=16384 tokens):
  x = x_c + x_t;  q/k/v = per-token linear projections of x
  per token: dot[h,g] = q_h . k_g * hd^-0.5   (8x8 gram ACROSS heads)
             attn = softmax_g(dot);  o_h = sum_g attn[h,g] v_g
  out = Wo @ o (+ bo)

Sharding: pure data parallel - one sample per NeuronCore, weights replicated.

Kernel design (per core, 32 groups of 4x128-token tiles). DVE is the
bottleneck engine, so everything that can leave it, does:
  - x-sum is folded into the QKV projection: ScalarE casts x_c/x_t tiles to
    fp16, PE projects BOTH against the same streamed [256,768] wqkv fp16
    weights, accumulating in PSUM (q/k rows host-permuted head-major, v in
    torch natural (c-major, g-fast) order).
  - one ACT copy per tile lands qkv in a [128, 3, G, 256] SBUF tile.
  - gram multiply + c-reduction binary tree + exp(ACT, scale folded) +
    denominator reduce + fast-approx reciprocal + attn-normalize on DVE,
    batched over the group via merged-stride APs; all fp16 2x mode.
  - numerator multiply (attn x v broadcast) writes pn [t, h, c, g] fp16;
    the g-CONTRACTION IS NOT DONE ON DVE: instead the output projection
    runs with a host-expanded Wo' [2048, 256] (Wo rows replicated 8x over
    g), so PE contracts (h,c,g)=2048 in 16 accumulating chunk-matmuls and
    the whole DVE g-tree disappears.
  - per tile, 16 PE transposes move pn to channel-major PSUM; ScalarE
    evacuates; out-projection -> fp32 PSUM -> ScalarE copy (+bo as per-
    partition ACT bias) -> 512-token DMA stores, channel-major.
Predicted (cost model, 0.96GHz DVE): ~505us DVE busy, ~490us PE, ~250us ACT.
"""

import os
import sys

if "/opt/trn_rl_repo" not in sys.path:
    sys.path.insert(0, "/opt/trn_rl_repo")

from contextlib import ExitStack

import numpy as np

import concourse.bacc as bacc
import concourse.tile as tile
from concourse import mybir
from concourse.bass_utils import run_bass_kernel_spmd
from concourse.masks import make_identity

B, C, HH, WW = 8, 256, 128, 128
NH, HD = 8, 32
N = HH * WW  # tokens per sample
TT = 128  # tokens per sub-tile (partition dim)
G = 4  # sub-tiles per group
GT = G * TT
GR = 4  # residual g extent entering the out-projection (DVE folds 8 -> GR)
KCH = (C * GR) // 128  # contraction chunks for the g-expanded out-proj
SCALE = float(HD) ** -0.5

F32 = mybir.dt.float32
F16 = mybir.dt.float16
AX = mybir.AxisListType


def build_kernel(n_tiles=N // TT, has_qkv_bias=False, has_o_bias=False):
    assert n_tiles % G == 0
    n_groups = n_tiles // G
    nc = bacc.Bacc(trn_type="TRN2")

    xc = nc.declare_dram_parameter("xc", [C, N], F32, isOutput=False)
    xt = nc.declare_dram_parameter("xt", [C, N], F32, isOutput=False)
    wqkv = nc.declare_dram_parameter("wqkv", [C, 3 * C], F16, isOutput=False)
    wox = nc.declare_dram_parameter("wox", [C * GR, C], F16, isOutput=False)
    bqkv = nc.declare_dram_parameter("bqkv", [1, 3 * C], F16, isOutput=False)
    bo = nc.declare_dram_parameter("bo", [C, 1], F32, isOutput=False)
    out = nc.declare_dram_parameter("out", [C, N], F32, isOutput=True)

    with tile.TileContext(nc) as tc, ExitStack() as ctx:
        singles = ctx.enter_context(tc.tile_pool(name="singles", bufs=1))
        xs_pool = ctx.enter_context(tc.tile_pool(name="xs", bufs=2))
        x16_pool = ctx.enter_context(tc.tile_pool(name="x16", bufs=2))
        qkv_pool = ctx.enter_context(tc.tile_pool(name="qkv", bufs=2))
        gp_pool = ctx.enter_context(tc.tile_pool(name="gp", bufs=2))
        pn_pool = ctx.enter_context(tc.tile_pool(name="pn", bufs=2))
        tree_pool = ctx.enter_context(tc.tile_pool(name="tree", bufs=1))
        tree2_pool = ctx.enter_context(tc.tile_pool(name="tree2", bufs=2))
        sm_pool = ctx.enter_context(tc.tile_pool(name="sm", bufs=2))
        otr_pool = ctx.enter_context(tc.tile_pool(name="otr", bufs=2))
        out_pool = ctx.enter_context(tc.tile_pool(name="outp", bufs=2))
        ps_qkv = ctx.enter_context(tc.tile_pool(name="ps_qkv", bufs=2, space="PSUM"))
        ps_tr = ctx.enter_context(tc.tile_pool(name="ps_tr", bufs=2, space="PSUM"))
        ps_out = ctx.enter_context(tc.tile_pool(name="ps_out", bufs=1, space="PSUM"))

        wqkv_sb = singles.tile([128, 2, 3 * C], F16)
        nc.sync.dma_start(out=wqkv_sb, in_=wqkv.rearrange("(b p) m -> p b m", b=2))
        wox_sb = singles.tile([128, KCH, C], F16)
        nc.sync.dma_start(out=wox_sb, in_=wox.rearrange("(k p) m -> p k m", k=KCH))
        ident = singles.tile([128, 128], F16)
        make_identity(nc, ident)
        if has_qkv_bias:
            bqkv_sb = singles.tile([1, 3 * C], F16)
            nc.sync.dma_start(out=bqkv_sb, in_=bqkv)
            ones_tok = singles.tile([1, TT], F16)
            nc.vector.memset(ones_tok, 1.0)
        if has_o_bias:
            bo_sb = singles.tile([128, 2], F32)
            nc.sync.dma_start(out=bo_sb, in_=bo.rearrange("(b p) o -> p (b o)", b=2))

        xc_r = xc.rearrange("(b p) n -> p b n", b=2)
        xt_r = xt.rearrange("(b p) n -> p b n", b=2)
        out_r = out.rearrange("(b p) n -> b p n", b=2)

        for g in range(n_groups):
            gtok = slice(g * GT, (g + 1) * GT)

            # ---- group load + fp16 casts (x-sum is folded into the matmuls)
            xc_t = xs_pool.tile([128, 2, GT], F32, tag="xc")
            xt_t = xs_pool.tile([128, 2, GT], F32, tag="xt")
            nc.sync.dma_start(out=xc_t, in_=xc_r[:, :, gtok])
            nc.sync.dma_start(out=xt_t, in_=xt_r[:, :, gtok])
            xc16 = x16_pool.tile([128, 2, GT], F16, tag="xc16")
            xt16 = x16_pool.tile([128, 2, GT], F16, tag="xt16")
            nc.scalar.copy(out=xc16, in_=xc_t)
            nc.scalar.copy(out=xt16, in_=xt_t)

            qkv = qkv_pool.tile([128, 3, G, C], F16, tag="qkv")
            gp = gp_pool.tile([128, G, NH, NH, HD], F16, tag="gp")
            pn = pn_pool.tile([128, G, NH, HD, NH], F16, tag="pn")

            # ---- per sub-tile qkv projection: q = Wq'(xc+xt) etc. via PSUM
            # accumulation over {xc,xt} x {channel-half b}; x-tile stationary.
            n_acc = 5 if has_qkv_bias else 4
            for j in range(G):
                qkv_ps = ps_qkv.tile([TT, 3 * C], F32, tag="qkvps")
                acc = 0
                for xsrc in (xc16, xt16):
                    for b in range(2):
                        lhs = xsrc[:, b, j * TT : (j + 1) * TT]
                        for lo, hi in ((0, 512), (512, 768)):
                            nc.tensor.matmul(
                                qkv_ps[:, lo:hi],
                                lhsT=lhs,
                                rhs=wqkv_sb[:, b, lo:hi],
                                start=(acc == 0),
                                stop=(acc == n_acc - 1),
                            )
                        acc += 1
                if has_qkv_bias:
                    for lo, hi in ((0, 512), (512, 768)):
                        nc.tensor.matmul(
                            qkv_ps[:, lo:hi],
                            lhsT=ones_tok,
                            rhs=bqkv_sb[:, lo:hi],
                            start=False,
                            stop=True,
                        )
                nc.scalar.copy(
                    out=qkv[:, :, j, :],
                    in_=qkv_ps.rearrange("p (r m) -> p r m", r=3),
                )

            # ---- group gram multiply: ONE DVE op via 4-free-dim broadcast APs
            q_b = (
                qkv[:, 0]
                .rearrange("p t (h c) -> p t h c", h=NH)
                .unsqueeze(3)
                .broadcast_to([TT, G, NH, NH, HD])
            )
            k_b = (
                qkv[:, 1]
                .rearrange("p t (g c) -> p t g c", g=NH)
                .unsqueeze(2)
                .broadcast_to([TT, G, NH, NH, HD])
            )
            nc.vector.tensor_mul(gp, q_b, k_b)

            # ---- grouped gram c-tree (merged-stride 3D APs over (t,h,g))
            gpv = gp.rearrange("p t h g c -> p (t h g) c")  # [128, 256, 32]
            t16 = tree_pool.tile([128, G * 64, 16], F16, tag="t16")
            nc.vector.tensor_add(t16, gpv[:, :, 0:16], gpv[:, :, 16:32])
            t8 = tree2_pool.tile([128, G * 64, 8], F16, tag="t8")
            nc.vector.tensor_add(t8, t16[:, :, 0:8], t16[:, :, 8:16])
            t4 = tree2_pool.tile([128, G * 64, 4], F16, tag="t4")
            nc.gpsimd.tensor_add(t4, t8[:, :, 0:4], t8[:, :, 4:8])
            t2 = tree2_pool.tile([128, G * 64, 2], F16, tag="t2")
            nc.gpsimd.tensor_add(t2, t4[:, :, 0:2], t4[:, :, 2:4])
            dot4 = sm_pool.tile([128, G, NH, NH], F16, tag="dot4")
            nc.gpsimd.tensor_add(
                dot4.rearrange("p t h g -> p (t h g)"), t2[:, :, 0], t2[:, :, 1]
            )

            # ---- grouped exp (ACT, softmax scale folded) + softmax tail (DVE)
            ex4 = sm_pool.tile([128, G, NH, NH], F16, tag="ex4")
            nc.scalar.activation(
                out=ex4.rearrange("p t h g -> p (t h g)"),
                in_=dot4.rearrange("p t h g -> p (t h g)"),
                func=mybir.ActivationFunctionType.Exp,
                scale=SCALE,
            )
            den4 = sm_pool.tile([128, G * NH], F32, tag="den4")
            nc.vector.reduce_sum(
                den4, ex4.rearrange("p t h g -> p (t h) g"), axis=AX.X
            )
            rec4 = sm_pool.tile([128, G * NH], F16, tag="rec4")
            with nc.allow_low_precision(reason="softmax weights tolerate fp16"):
                nc.vector.reciprocal(rec4, den4)
            attn4 = sm_pool.tile([128, G, NH, NH], F16, tag="attn4")
            nc.gpsimd.tensor_mul(
                attn4.rearrange("p t h g -> p (t h) g"),
                ex4.rearrange("p t h g -> p (t h) g"),
                rec4.unsqueeze(2).broadcast_to([128, G * NH, NH]),
            )

            # ---- group numerator multiply: ONE DVE op (attn x v broadcast).
            # pn [t, h, c, g8]; one DVE tree level folds g to GR=4, the
            # remaining g-sum happens on PE inside the g-expanded out-proj.
            a_b = attn4.unsqueeze(3).broadcast_to([TT, G, NH, HD, NH])
            v_b = (
                qkv[:, 2]
                .rearrange("p t (c g) -> p t c g", g=NH)
                .unsqueeze(2)
                .broadcast_to([TT, G, NH, HD, NH])
            )
            nc.vector.tensor_mul(pn, a_b, v_b)
            pn4 = pn_pool.tile([128, G, NH, HD, GR], F16, tag="pn4")
            pnv = pn.rearrange("p t h c g -> p (t h c) g")
            nc.vector.tensor_add(
                pn4.rearrange("p t h c g -> p (t h c) g"),
                pnv[:, :, 0:GR],
                pnv[:, :, GR : 2 * GR],
            )

            # ---- per sub-tile: KCH PE transposes pn4 -> channel-major PSUM,
            # one ACT evacuation per tile into the group otr buffer.
            otr_sb = otr_pool.tile([128, KCH, GT], F16, tag="otrsb")
            for j in range(G):
                pnf = pn4[:, j].rearrange("p h c g -> p (h c g)")  # [TT, 1024]
                tr_ps = ps_tr.tile([128, KCH, TT], F16, tag="trps")
                for ci in range(KCH):
                    nc.tensor.transpose(
                        tr_ps[:, ci], pnf[:, ci * 128 : (ci + 1) * 128], ident
                    )
                nc.scalar.copy(
                    out=otr_sb[:, :, j * TT : (j + 1) * TT], in_=tr_ps
                )

            # ---- out-projection with g-expanded Wo': contracts (h,c,g)=2048
            # in 16 accumulating chunk-matmuls per 128-channel output half.
            out_ps = ps_out.tile([128, 2, GT], F32, tag="outps")
            for m in range(2):
                for ci in range(KCH):
                    nc.tensor.matmul(
                        out_ps[:, m],
                        lhsT=wox_sb[:, ci, m * 128 : (m + 1) * 128],
                        rhs=otr_sb[:, ci],
                        start=(ci == 0),
                        stop=(ci == KCH - 1),
                    )
            out_sb = out_pool.tile([128, 2, GT], F32, tag="outsb")
            if has_o_bias:
                for m in range(2):
                    nc.scalar.activation(
                        out=out_sb[:, m],
                        in_=out_ps[:, m],
                        func=mybir.ActivationFunctionType.Identity,
                        bias=bo_sb[:, m : m + 1],
                    )
            else:
                nc.scalar.copy(out=out_sb, in_=out_ps)
            for b in range(2):
                nc.sync.dma_start(out=out_r[b, :, gtok], in_=out_sb[:, b])

    nc.compile()
    return nc


# split_heads uses channel c*nh+h for (head h, dim c); permute projection rows
# so q,k come out head-major (h*32+c). v stays in natural order: its (c,g)
# interleave is exactly what the numerator multiply wants. The out-proj weight
# is expanded over g: pn layout is [h, c, g] so row (h*32+c)*8+g of Wo' equals
# WoT row h*32+c (Wo's input channels already match head-major merge order).
_PERM = np.array([c * NH + h for h in range(NH) for c in range(HD)])


def _prep_weights(Wq, bq, Wk, bk, Wv, bv, Wo, bo):
    wqkv = np.concatenate([Wq[_PERM].T, Wk[_PERM].T, Wv.T], axis=1)
    wqkv = np.ascontiguousarray(wqkv).astype(np.float16)
    wot = np.ascontiguousarray(Wo.T)  # [256 (h,c), 256 m]
    wox = np.repeat(wot, 4, axis=0).astype(np.float16)  # [(h,c,g4)=1024, 256]
    bqkv = np.concatenate([bq[_PERM], bk[_PERM], bv]).astype(np.float16)[None]
    bo_a = bo.astype(np.float32)[:, None]
    return wqkv, wox, bqkv, bo_a


def _in_maps(x_c, x_t, wqkv, wox, bqkv, bo_a):
    maps = []
    for b in range(B):
        maps.append(
            {
                "xc": np.ascontiguousarray(x_c[b].reshape(C, N)),
                "xt": np.ascontiguousarray(x_t[b].reshape(C, N)),
                "wqkv": wqkv,
                "wox": wox,
                "bqkv": bqkv,
                "bo": bo_a,
            }
        )
    return maps


def kernel(x_c, x_t, Wq, bq, Wk, bk, Wv, bv, Wo, bo):
    x_c = np.asarray(x_c, dtype=np.float32)
    x_t = np.asarray(x_t, dtype=np.float32)
    wqkv, wox, bqkv, bo_a = _prep_weights(
        np.asarray(Wq, np.float32),
        np.asarray(bq, np.float32),
        np.asarray(Wk, np.float32),
        np.asarray(bk, np.float32),
        np.asarray(Wv, np.float32),
        np.asarray(bv, np.float32),
        np.asarray(Wo, np.float32),
        np.asarray(bo, np.float32),
    )
    nc = build_kernel(
        has_qkv_bias=bool(np.any(bqkv)), has_o_bias=bool(np.any(bo_a))
    )
    res = run_bass_kernel_spmd(
        nc, _in_maps(x_c, x_t, wqkv, wox, bqkv, bo_a), list(range(B))
    )
    outs = [np.asarray(res.results[b]["out"]).reshape(C, HH, WW) for b in range(B)]
    return np.stack(outs).astype(np.float32)


def _install_ntff_shim():
    """Recreate the missing antenv.axon_hooks module + ctypes NTFF hook
    (mirrors trn_agent_boot.trn_boot's degraded-silently path). Test-only."""
    import contextlib
    import ctypes
    import types

    try:
        from antenv.axon_hooks import get_axon_ntff_profile_hook  # noqa: F401

        return True
    except ImportError:
        pass
    import antenv

    mod = types.ModuleType("antenv.axon_hooks")
    mod._hook = None

    def set_axon_ntff_profile_hook(h):
        mod._hook = h

    def get_axon_ntff_profile_hook():
        return mod._hook

    mod.set_axon_ntff_profile_hook = set_axon_ntff_profile_hook
    mod.get_axon_ntff_profile_hook = get_axon_ntff_profile_hook
    sys.modules["antenv.axon_hooks"] = mod
    antenv.axon_hooks = mod

    so_path = "/opt/axon/libaxon_pjrt.so"
    if not os.path.exists(so_path):
        return False
    lib = ctypes.CDLL(so_path)
    if not hasattr(lib, "axon_start_nrt_profile"):
        return False
    lib.axon_start_nrt_profile.argtypes = [
        ctypes.POINTER(ctypes.c_int64),
        ctypes.c_size_t,
    ]
    lib.axon_start_nrt_profile.restype = ctypes.c_int64
    lib.axon_stop_nrt_profile.argtypes = [ctypes.c_char_p]
    lib.axon_stop_nrt_profile.restype = ctypes.c_int64

    @contextlib.contextmanager
    def _hook(output_dir, device_ids):
        import jax

        jax.devices()
        if device_ids:
            ids = (ctypes.c_int64 * len(device_ids))(*device_ids)
            rc = lib.axon_start_nrt_profile(ids, len(device_ids))
        else:
            rc = lib.axon_start_nrt_profile(None, 0)
        if rc != 0:
            raise RuntimeError(f"axon_start_nrt_profile rc={rc}")
        try:
            yield
        finally:
            n = lib.axon_stop_nrt_profile(str(output_dir).encode())
            print(f"profile: {n} file(s) written to {output_dir}")

    set_axon_ntff_profile_hook(_hook)
    return True


def profile_run(inputs_np):
    """Run once more with NTFF tracing on core 0; return exec_time_ns."""
    import concourse.bass_utils as bu

    _install_ntff_shim()
    bu.upload_artifacts = lambda d: "local://" + d  # no S3 in this container
    x_c = np.asarray(inputs_np["x_c"], np.float32)
    x_t = np.asarray(inputs_np["x_t"], np.float32)
    wqkv, wox, bqkv, bo_a = _prep_weights(
        *[
            np.asarray(inputs_np[k], np.float32)
            for k in ("Wq", "bq", "Wk", "bk", "Wv", "bv", "Wo", "bo")
        ]
    )
    nc = build_kernel(
        has_qkv_bias=bool(np.any(bqkv)), has_o_bias=bool(np.any(bo_a))
    )
    res = run_bass_kernel_spmd(
        nc,
        _in_maps(x_c, x_t, wqkv, wox, bqkv, bo_a),
        list(range(B)),
        trace=True,
    )
    return res.exec_time_ns


if __name__ == "__main__":
    rng = np.random.default_rng(0)
    ins = {
        "x_c": rng.standard_normal((B, C, HH, WW), dtype=np.float32),
        "x_t": rng.standard_normal((B, C, HH, WW), dtype=np.float32),
        "Wq": (rng.standard_normal((C, C)) * 0.02).astype(np.float32),
        "bq": np.zeros(C, np.float32),
        "Wk": (rng.standard_normal((C, C)) * 0.02).astype(np.float32),
        "bk": np.zeros(C, np.float32),
        "Wv": (rng.standard_normal((C, C)) * 0.02).astype(np.float32),
        "bv": np.zeros(C, np.float32),
        "Wo": (rng.standard_normal((C, C)) * 0.02).astype(np.float32),
        "bo": np.zeros(C, np.float32),
    }
    out = kernel(**ins)
    print(out.shape, out.dtype)


# revision 15
# speedup vs baseline: 1.1466x; 1.1466x over previous
"""Trainium2 Bass kernel: fused cross-head attention block (data parallel, 8 cores).

Problem (hardcoded shapes): x_c, x_t [8, 256, 128, 128] f32; Wq/Wk/Wv/Wo
[256, 256]; biases [256]; out [8, 256, 128, 128] f32.

Math per sample (C=256, nh=8, hd=32, N=H*W=16384 tokens):
  x = x_c + x_t;  q/k/v = per-token linear projections of x
  per token: dot[h,g] = q_h . k_g * hd^-0.5   (8x8 gram ACROSS heads)
             attn = softmax_g(dot);  o_h = sum_g attn[h,g] v_g
  out = Wo @ o (+ bo)

Sharding: pure data parallel - one sample per NeuronCore, weights replicated.

Kernel design (per core, 32 groups of 4x128-token tiles). DVE is the
bottleneck engine, so everything that can leave it, does:
  - x-sum is folded into the QKV projection: ScalarE casts x_c/x_t tiles to
    fp16, PE projects BOTH against the same streamed [256,768] wqkv fp16
    weights, accumulating in PSUM (q/k rows host-permuted head-major, v in
    torch natural (c-major, g-fast) order).
  - one ACT copy per tile lands qkv in a [128, 3, G, 256] SBUF tile.
  - gram multiply + c-reduction binary tree + exp(ACT, scale folded) +
    denominator reduce + fast-approx reciprocal + attn-normalize on DVE,
    batched over the group via merged-stride APs; all fp16 2x mode.
  - numerator multiply (attn x v broadcast) writes pn [t, h, c, g] fp16;
    the g-CONTRACTION IS NOT DONE ON DVE: instead the output projection
    runs with a host-expanded Wo' [2048, 256] (Wo rows replicated 8x over
    g), so PE contracts (h,c,g)=2048 in 16 accumulating chunk-matmuls and
    the whole DVE g-tree disappears.
  - per tile, 16 PE transposes move pn to channel-major PSUM; ScalarE
    evacuates; out-projection -> fp32 PSUM -> ScalarE copy (+bo as per-
    partition ACT bias) -> 512-token DMA stores, channel-major.
Predicted (cost model, 0.96GHz DVE): ~505us DVE busy, ~490us PE, ~250us ACT.
"""

import os
import sys

if "/opt/trn_rl_repo" not in sys.path:
    sys.path.insert(0, "/opt/trn_rl_repo")

from contextlib import ExitStack

import numpy as np

import concourse.bacc as bacc
import concourse.tile as tile
from concourse import mybir
from concourse.bass_utils import run_bass_kernel_spmd
from concourse.masks import make_identity

B, C, HH, WW = 8, 256, 128, 128
NH, HD = 8, 32
N = HH * WW  # tokens per sample
TT = 128  # tokens per sub-tile (partition dim)
G = 4  # sub-tiles per group
GT = G * TT
GR = 4  # residual g extent entering the out-projection (DVE folds 8 -> GR)
KCH = (C * GR) // 128  # contraction chunks for the g-expanded out-proj
SCALE = float(HD) ** -0.5

F32 = mybir.dt.float32
F16 = mybir.dt.float16
AX = mybir.AxisListType


def build_kernel(n_tiles=N // TT, has_qkv_bias=False, has_o_bias=False):
    assert n_tiles % G == 0
    n_groups = n_tiles // G
    nc = bacc.Bacc(trn_type="TRN2")

    xc = nc.declare_dram_parameter("xc", [C, N], F32, isOutput=False)
    xt = nc.declare_dram_parameter("xt", [C, N], F32, isOutput=False)
    wqkv = nc.declare_dram_parameter("wqkv", [C, 3 * C], F16, isOutput=False)
    wox = nc.declare_dram_parameter("wox", [C * GR, C], F16, isOutput=False)
    bqkv = nc.declare_dram_parameter("bqkv", [1, 3 * C], F16, isOutput=False)
    bo = nc.declare_dram_parameter("bo", [C, 1], F32, isOutput=False)
    out = nc.declare_dram_parameter("out", [C, N], F32, isOutput=True)

    with tile.TileContext(nc) as tc, ExitStack() as ctx:
        singles = ctx.enter_context(tc.tile_pool(name="singles", bufs=1))
        xs_pool = ctx.enter_context(tc.tile_pool(name="xs", bufs=2))
        x16_pool = ctx.enter_context(tc.tile_pool(name="x16", bufs=2))
        qkv_pool = ctx.enter_context(tc.tile_pool(name="qkv", bufs=2))
        gp_pool = ctx.enter_context(tc.tile_pool(name="gp", bufs=2))
        pn_pool = ctx.enter_context(tc.tile_pool(name="pn", bufs=2))
        tree_pool = ctx.enter_context(tc.tile_pool(name="tree", bufs=1))
        tree2_pool = ctx.enter_context(tc.tile_pool(name="tree2", bufs=2))
        sm_pool = ctx.enter_context(tc.tile_pool(name="sm", bufs=2))
        otr_pool = ctx.enter_context(tc.tile_pool(name="otr", bufs=2))
        out_pool = ctx.enter_context(tc.tile_pool(name="outp", bufs=2))
        ps_qkv = ctx.enter_context(tc.tile_pool(name="ps_qkv", bufs=2, space="PSUM"))
        ps_tr = ctx.enter_context(tc.tile_pool(name="ps_tr", bufs=2, space="PSUM"))
        ps_out = ctx.enter_context(tc.tile_pool(name="ps_out", bufs=1, space="PSUM"))

        wqkv_sb = singles.tile([128, 2, 3 * C], F16)
        nc.sync.dma_start(out=wqkv_sb, in_=wqkv.rearrange("(b p) m -> p b m", b=2))
        wox_sb = singles.tile([128, KCH, C], F16)
        nc.sync.dma_start(out=wox_sb, in_=wox.rearrange("(k p) m -> p k m", k=KCH))
        ident = singles.tile([128, 128], F16)
        make_identity(nc, ident)
        if has_qkv_bias:
            bqkv_sb = singles.tile([1, 3 * C], F16)
            nc.sync.dma_start(out=bqkv_sb, in_=bqkv)
            ones_tok = singles.tile([1, TT], F16)
            nc.vector.memset(ones_tok, 1.0)
        if has_o_bias:
            bo_sb = singles.tile([128, 2], F32)
            nc.sync.dma_start(out=bo_sb, in_=bo.rearrange("(b p) o -> p (b o)", b=2))

        xc_r = xc.rearrange("(b p) n -> p b n", b=2)
        xt_r = xt.rearrange("(b p) n -> p b n", b=2)
        out_r = out.rearrange("(b p) n -> b p n", b=2)

        for g in range(n_groups):
            gtok = slice(g * GT, (g + 1) * GT)

            # ---- group load + fp16 casts (x-sum is folded into the matmuls)
            xc_t = xs_pool.tile([128, 2, GT], F32, tag="xc")
            xt_t = xs_pool.tile([128, 2, GT], F32, tag="xt")
            nc.sync.dma_start(out=xc_t, in_=xc_r[:, :, gtok])
            nc.sync.dma_start(out=xt_t, in_=xt_r[:, :, gtok])
            xc16 = x16_pool.tile([128, 2, GT], F16, tag="xc16")
            xt16 = x16_pool.tile([128, 2, GT], F16, tag="xt16")
            nc.scalar.copy(out=xc16, in_=xc_t)
            nc.scalar.copy(out=xt16, in_=xt_t)

            qkv = qkv_pool.tile([128, 3, G, C], F16, tag="qkv")
            gp = gp_pool.tile([128, G, NH, NH, HD], F16, tag="gp")
            pn = pn_pool.tile([128, G, NH, HD, NH], F16, tag="pn")

            # ---- per sub-tile qkv projection: q = Wq'(xc+xt) etc. via PSUM
            # accumulation over {xc,xt} x {channel-half b}; x-tile stationary.
            n_acc = 5 if has_qkv_bias else 4
            for j in range(G):
                qkv_ps = ps_qkv.tile([TT, 3 * C], F32, tag="qkvps")
                acc = 0
                for xsrc in (xc16, xt16):
                    for b in range(2):
                        lhs = xsrc[:, b, j * TT : (j + 1) * TT]
                        for lo, hi in ((0, 512), (512, 768)):
                            nc.tensor.matmul(
                                qkv_ps[:, lo:hi],
                                lhsT=lhs,
                                rhs=wqkv_sb[:, b, lo:hi],
                                start=(acc == 0),
                                stop=(acc == n_acc - 1),
                            )
                        acc += 1
                if has_qkv_bias:
                    for lo, hi in ((0, 512), (512, 768)):
                        nc.tensor.matmul(
                            qkv_ps[:, lo:hi],
                            lhsT=ones_tok,
                            rhs=bqkv_sb[:, lo:hi],
                            start=False,
                            stop=True,
                        )
                nc.scalar.copy(
                    out=qkv[:, :, j, :],
                    in_=qkv_ps.rearrange("p (r m) -> p r m", r=3),
                )

            # ---- group gram multiply: ONE DVE op via 4-free-dim broadcast APs
            q_b = (
                qkv[:, 0]
                .rearrange("p t (h c) -> p t h c", h=NH)
                .unsqueeze(3)
                .broadcast_to([TT, G, NH, NH, HD])
            )
            k_b = (
                qkv[:, 1]
                .rearrange("p t (g c) -> p t g c", g=NH)
                .unsqueeze(2)
                .broadcast_to([TT, G, NH, NH, HD])
            )
            nc.vector.tensor_mul(gp, q_b, k_b)

            # ---- grouped gram c-tree (merged-stride 3D APs over (t,h,g))
            gpv = gp.rearrange("p t h g c -> p (t h g) c")  # [128, 256, 32]
            t16 = tree_pool.tile([128, G * 64, 16], F16, tag="t16")
            nc.vector.tensor_add(t16, gpv[:, :, 0:16], gpv[:, :, 16:32])
            t8 = tree2_pool.tile([128, G * 64, 8], F16, tag="t8")
            nc.vector.tensor_add(t8, t16[:, :, 0:8], t16[:, :, 8:16])
            t4 = tree2_pool.tile([128, G * 64, 4], F16, tag="t4")
            nc.vector.tensor_add(t4, t8[:, :, 0:4], t8[:, :, 4:8])
            t2 = tree2_pool.tile([128, G * 64, 2], F16, tag="t2")
            nc.vector.tensor_add(t2, t4[:, :, 0:2], t4[:, :, 2:4])
            dot4 = sm_pool.tile([128, G, NH, NH], F16, tag="dot4")
            nc.vector.tensor_add(
                dot4.rearrange("p t h g -> p (t h g)"), t2[:, :, 0], t2[:, :, 1]
            )

            # ---- grouped exp (ACT, softmax scale folded) + softmax tail (DVE)
            ex4 = sm_pool.tile([128, G, NH, NH], F16, tag="ex4")
            nc.scalar.activation(
                out=ex4.rearrange("p t h g -> p (t h g)"),
                in_=dot4.rearrange("p t h g -> p (t h g)"),
                func=mybir.ActivationFunctionType.Exp,
                scale=SCALE,
            )
            den4 = sm_pool.tile([128, G * NH], F32, tag="den4")
            nc.vector.reduce_sum(
                den4, ex4.rearrange("p t h g -> p (t h) g"), axis=AX.X
            )
            rec4 = sm_pool.tile([128, G * NH], F16, tag="rec4")
            with nc.allow_low_precision(reason="softmax weights tolerate fp16"):
                nc.vector.reciprocal(rec4, den4)
            attn4 = sm_pool.tile([128, G, NH, NH], F16, tag="attn4")
            nc.vector.tensor_mul(
                attn4.rearrange("p t h g -> p (t h) g"),
                ex4.rearrange("p t h g -> p (t h) g"),
                rec4.unsqueeze(2).broadcast_to([128, G * NH, NH]),
            )

            # ---- group numerator multiply: ONE DVE op (attn x v broadcast).
            # pn [t, h, c, g8]; one DVE tree level folds g to GR=4, the
            # remaining g-sum happens on PE inside the g-expanded out-proj.
            a_b = attn4.unsqueeze(3).broadcast_to([TT, G, NH, HD, NH])
            v_b = (
                qkv[:, 2]
                .rearrange("p t (c g) -> p t c g", g=NH)
                .unsqueeze(2)
                .broadcast_to([TT, G, NH, HD, NH])
            )
            nc.vector.tensor_mul(pn, a_b, v_b)
            pn4 = pn_pool.tile([128, G, NH, HD, GR], F16, tag="pn4")
            pnv = pn.rearrange("p t h c g -> p (t h c) g")
            nc.vector.tensor_add(
                pn4.rearrange("p t h c g -> p (t h c) g"),
                pnv[:, :, 0:GR],
                pnv[:, :, GR : 2 * GR],
            )

            # ---- per sub-tile: KCH PE transposes pn4 -> channel-major PSUM,
            # one ACT evacuation per tile into the group otr buffer.
            otr_sb = otr_pool.tile([128, KCH, GT], F16, tag="otrsb")
            for j in range(G):
                pnf = pn4[:, j].rearrange("p h c g -> p (h c g)")  # [TT, 1024]
                tr_ps = ps_tr.tile([128, KCH, TT], F16, tag="trps")
                for ci in range(KCH):
                    nc.tensor.transpose(
                        tr_ps[:, ci], pnf[:, ci * 128 : (ci + 1) * 128], ident
                    )
                nc.scalar.copy(
                    out=otr_sb[:, :, j * TT : (j + 1) * TT], in_=tr_ps
                )

            # ---- out-projection with g-expanded Wo': contracts (h,c,g)=2048
            # in 16 accumulating chunk-matmuls per 128-channel output half.
            out_ps = ps_out.tile([128, 2, GT], F32, tag="outps")
            for m in range(2):
                for ci in range(KCH):
                    nc.tensor.matmul(
                        out_ps[:, m],
                        lhsT=wox_sb[:, ci, m * 128 : (m + 1) * 128],
                        rhs=otr_sb[:, ci],
                        start=(ci == 0),
                        stop=(ci == KCH - 1),
                    )
            out_sb = out_pool.tile([128, 2, GT], F32, tag="outsb")
            if has_o_bias:
                for m in range(2):
                    nc.scalar.activation(
                        out=out_sb[:, m],
                        in_=out_ps[:, m],
                        func=mybir.ActivationFunctionType.Identity,
                        bias=bo_sb[:, m : m + 1],
                    )
            else:
                nc.scalar.copy(out=out_sb, in_=out_ps)
            for b in range(2):
                nc.sync.dma_start(out=out_r[b, :, gtok], in_=out_sb[:, b])

    nc.compile()
    return nc


# split_heads uses channel c*nh+h for (head h, dim c); permute projection rows
# so q,k come out head-major (h*32+c). v stays in natural order: its (c,g)
# interleave is exactly what the numerator multiply wants. The out-proj weight
# is expanded over g: pn layout is [h, c, g] so row (h*32+c)*8+g of Wo' equals
# WoT row h*32+c (Wo's input channels already match head-major merge order).
_PERM = np.array([c * NH + h for h in range(NH) for c in range(HD)])


def _prep_weights(Wq, bq, Wk, bk, Wv, bv, Wo, bo):
    wqkv = np.concatenate([Wq[_PERM].T, Wk[_PERM].T, Wv.T], axis=1)
    wqkv = np.ascontiguousarray(wqkv).astype(np.float16)
    wot = np.ascontiguousarray(Wo.T)  # [256 (h,c), 256 m]
    wox = np.repeat(wot, 4, axis=0).astype(np.float16)  # [(h,c,g4)=1024, 256]
    bqkv = np.concatenate([bq[_PERM], bk[_PERM], bv]).astype(np.float16)[None]
    bo_a = bo.astype(np.float32)[:, None]
    return wqkv, wox, bqkv, bo_a


def _in_maps(x_c, x_t, wqkv, wox, bqkv, bo_a):
    maps = []
    for b in range(B):
        maps.append(
            {
                "xc": np.ascontiguousarray(x_c[b].reshape(C, N)),
                "xt": np.ascontiguousarray(x_t[b].reshape(C, N)),
                "wqkv": wqkv,
                "wox": wox,
                "bqkv": bqkv,
                "bo": bo_a,
            }
        )
    return maps


def kernel(x_c, x_t, Wq, bq, Wk, bk, Wv, bv, Wo, bo):
    x_c = np.asarray(x_c, dtype=np.float32)
    x_t = np.asarray(x_t, dtype=np.float32)
    wqkv, wox, bqkv, bo_a = _prep_weights(
        np.asarray(Wq, np.float32),
        np.asarray(bq, np.float32),
        np.asarray(Wk, np.float32),
        np.asarray(bk, np.float32),
        np.asarray(Wv, np.float32),
        np.asarray(bv, np.float32),
        np.asarray(Wo, np.float32),
        np.asarray(bo, np.float32),
    )
    nc = build_kernel(
        has_qkv_bias=bool(np.any(bqkv)), has_o_bias=bool(np.any(bo_a))
    )
    res = run_bass_kernel_spmd(
        nc, _in_maps(x_c, x_t, wqkv, wox, bqkv, bo_a), list(range(B))
    )
    outs = [np.asarray(res.results[b]["out"]).reshape(C, HH, WW) for b in range(B)]
    return np.stack(outs).astype(np.float32)


def _install_ntff_shim():
    """Recreate the missing antenv.axon_hooks module + ctypes NTFF hook
    (mirrors trn_agent_boot.trn_boot's degraded-silently path). Test-only."""
    import contextlib
    import ctypes
    import types

    try:
        from antenv.axon_hooks import get_axon_ntff_profile_hook  # noqa: F401

        return True
    except ImportError:
        pass
    import antenv

    mod = types.ModuleType("antenv.axon_hooks")
    mod._hook = None

    def set_axon_ntff_profile_hook(h):
        mod._hook = h

    def get_axon_ntff_profile_hook():
        return mod._hook

    mod.set_axon_ntff_profile_hook = set_axon_ntff_profile_hook
    mod.get_axon_ntff_profile_hook = get_axon_ntff_profile_hook
    sys.modules["antenv.axon_hooks"] = mod
    antenv.axon_hooks = mod

    so_path = "/opt/axon/libaxon_pjrt.so"
    if not os.path.exists(so_path):
        return False
    lib = ctypes.CDLL(so_path)
    if not hasattr(lib, "axon_start_nrt_profile"):
        return False
    lib.axon_start_nrt_profile.argtypes = [
        ctypes.POINTER(ctypes.c_int64),
        ctypes.c_size_t,
    ]
    lib.axon_start_nrt_profile.restype = ctypes.c_int64
    lib.axon_stop_nrt_profile.argtypes = [ctypes.c_char_p]
    lib.axon_stop_nrt_profile.restype = ctypes.c_int64

    @contextlib.contextmanager
    def _hook(output_dir, device_ids):
        import jax

        jax.devices()
        if device_ids:
            ids = (ctypes.c_int64 * len(device_ids))(*device_ids)
            rc = lib.axon_start_nrt_profile(ids, len(device_ids))
        else:
            rc = lib.axon_start_nrt_profile(None, 0)
        if rc != 0:
            raise RuntimeError(f"axon_start_nrt_profile rc={rc}")
        try:
            yield
        finally:
            n = lib.axon_stop_nrt_profile(str(output_dir).encode())
            print(f"profile: {n} file(s) written to {output_dir}")

    set_axon_ntff_profile_hook(_hook)
    return True


def profile_run(inputs_np):
    """Run once more with NTFF tracing on core 0; return exec_time_ns."""
    import concourse.bass_utils as bu

    _install_ntff_shim()
    bu.upload_artifacts = lambda d: "local://" + d  # no S3 in this container
    x_c = np.asarray(inputs_np["x_c"], np.float32)
    x_t = np.asarray(inputs_np["x_t"], np.float32)
    wqkv, wox, bqkv, bo_a = _prep_weights(
        *[
            np.asarray(inputs_np[k], np.float32)
            for k in ("Wq", "bq", "Wk", "bk", "Wv", "bv", "Wo", "bo")
        ]
    )
    nc = build_kernel(
        has_qkv_bias=bool(np.any(bqkv)), has_o_bias=bool(np.any(bo_a))
    )
    res = run_bass_kernel_spmd(
        nc,
        _in_maps(x_c, x_t, wqkv, wox, bqkv, bo_a),
        list(range(B)),
        trace=True,
    )
    return res.exec_time_ns


if __name__ == "__main__":
    rng = np.random.default_rng(0)
    ins = {
        "x_c": rng.standard_normal((B, C, HH, WW), dtype=np.float32),
        "x_t": rng.standard_normal((B, C, HH, WW), dtype=np.float32),
        "Wq": (rng.standard_normal((C, C)) * 0.02).astype(np.float32),
        "bq": np.zeros(C, np.float32),
        "Wk": (rng.standard_normal((C, C)) * 0.02).astype(np.float32),
        "bk": np.zeros(C, np.float32),
        "Wv": (rng.standard_normal((C, C)) * 0.02).astype(np.float32),
        "bv": np.zeros(C, np.float32),
        "Wo": (rng.standard_normal((C, C)) * 0.02).astype(np.float32),
        "bo": np.zeros(C, np.float32),
    }
    out = kernel(**ins)
    print(out.shape, out.dtype)


# revision 16
# speedup vs baseline: 1.1547x; 1.0071x over previous
"""Trainium2 Bass kernel: fused cross-head attention block (data parallel, 8 cores).

Problem (hardcoded shapes): x_c, x_t [8, 256, 128, 128] f32; Wq/Wk/Wv/Wo
[256, 256]; biases [256]; out [8, 256, 128, 128] f32.

Math per sample (C=256, nh=8, hd=32, N=H*W=16384 tokens):
  x = x_c + x_t;  q/k/v = per-token linear projections of x
  per token: dot[h,g] = q_h . k_g * hd^-0.5   (8x8 gram ACROSS heads)
             attn = softmax_g(dot);  o_h = sum_g attn[h,g] v_g
  out = Wo @ o (+ bo)

Sharding: pure data parallel - one sample per NeuronCore, weights replicated.

Kernel design (per core, 32 groups of 4x128-token tiles). DVE is the
bottleneck engine, so everything that can leave it, does:
  - x-sum is folded into the QKV projection: ScalarE casts x_c/x_t tiles to
    fp16, PE projects BOTH against the same streamed [256,768] wqkv fp16
    weights, accumulating in PSUM (q/k rows host-permuted head-major, v in
    torch natural (c-major, g-fast) order).
  - one ACT copy per tile lands qkv in a [128, 3, G, 256] SBUF tile.
  - gram multiply + c-reduction binary tree + exp(ACT, scale folded) +
    denominator reduce + fast-approx reciprocal + attn-normalize on DVE,
    batched over the group via merged-stride APs; all fp16 2x mode.
  - numerator multiply (attn x v broadcast) writes pn [t, h, c, g] fp16;
    the g-CONTRACTION IS NOT DONE ON DVE: instead the output projection
    runs with a host-expanded Wo' [2048, 256] (Wo rows replicated 8x over
    g), so PE contracts (h,c,g)=2048 in 16 accumulating chunk-matmuls and
    the whole DVE g-tree disappears.
  - per tile, 16 PE transposes move pn to channel-major PSUM; ScalarE
    evacuates; out-projection -> fp32 PSUM -> ScalarE copy (+bo as per-
    partition ACT bias) -> 512-token DMA stores, channel-major.
Predicted (cost model, 0.96GHz DVE): ~505us DVE busy, ~490us PE, ~250us ACT.
"""

import os
import sys

if "/opt/trn_rl_repo" not in sys.path:
    sys.path.insert(0, "/opt/trn_rl_repo")

from contextlib import ExitStack

import numpy as np

import concourse.bacc as bacc
import concourse.tile as tile
from concourse import mybir
from concourse.bass_utils import run_bass_kernel_spmd
from concourse.masks import make_identity

B, C, HH, WW = 8, 256, 128, 128
NH, HD = 8, 32
N = HH * WW  # tokens per sample
TT = 128  # tokens per sub-tile (partition dim)
G = 4  # sub-tiles per group
GT = G * TT
GR = 4  # residual g extent entering the out-projection (DVE folds 8 -> GR)
KCH = (C * GR) // 128  # contraction chunks for the g-expanded out-proj
SCALE = float(HD) ** -0.5

F32 = mybir.dt.float32
F16 = mybir.dt.float16
AX = mybir.AxisListType


def build_kernel(n_tiles=N // TT, has_qkv_bias=False, has_o_bias=False):
    assert n_tiles % G == 0
    n_groups = n_tiles // G
    nc = bacc.Bacc(trn_type="TRN2")

    xc = nc.declare_dram_parameter("xc", [C, N], F32, isOutput=False)
    xt = nc.declare_dram_parameter("xt", [C, N], F32, isOutput=False)
    wqkv = nc.declare_dram_parameter("wqkv", [C, 3 * C], F16, isOutput=False)
    wox = nc.declare_dram_parameter("wox", [C * GR, C], F16, isOutput=False)
    bqkv = nc.declare_dram_parameter("bqkv", [1, 3 * C], F16, isOutput=False)
    bo = nc.declare_dram_parameter("bo", [C, 1], F32, isOutput=False)
    out = nc.declare_dram_parameter("out", [C, N], F32, isOutput=True)

    with tile.TileContext(nc) as tc, ExitStack() as ctx:
        singles = ctx.enter_context(tc.tile_pool(name="singles", bufs=1))
        xs_pool = ctx.enter_context(tc.tile_pool(name="xs", bufs=2))
        x16_pool = ctx.enter_context(tc.tile_pool(name="x16", bufs=2))
        qkv_pool = ctx.enter_context(tc.tile_pool(name="qkv", bufs=2))
        gp_pool = ctx.enter_context(tc.tile_pool(name="gp", bufs=2))
        pn_pool = ctx.enter_context(tc.tile_pool(name="pn", bufs=2))
        tree_pool = ctx.enter_context(tc.tile_pool(name="tree", bufs=1))
        tree2_pool = ctx.enter_context(tc.tile_pool(name="tree2", bufs=2))
        sm_pool = ctx.enter_context(tc.tile_pool(name="sm", bufs=2))
        otr_pool = ctx.enter_context(tc.tile_pool(name="otr", bufs=2))
        out_pool = ctx.enter_context(tc.tile_pool(name="outp", bufs=2))
        ps_qkv = ctx.enter_context(tc.tile_pool(name="ps_qkv", bufs=2, space="PSUM"))
        ps_tr = ctx.enter_context(tc.tile_pool(name="ps_tr", bufs=2, space="PSUM"))
        ps_out = ctx.enter_context(tc.tile_pool(name="ps_out", bufs=1, space="PSUM"))

        wqkv_sb = singles.tile([128, 2, 3 * C], F16)
        nc.sync.dma_start(out=wqkv_sb, in_=wqkv.rearrange("(b p) m -> p b m", b=2))
        wox_sb = singles.tile([128, KCH, C], F16)
        nc.sync.dma_start(out=wox_sb, in_=wox.rearrange("(k p) m -> p k m", k=KCH))
        ident = singles.tile([128, 128], F16)
        make_identity(nc, ident)
        if has_qkv_bias:
            bqkv_sb = singles.tile([1, 3 * C], F16)
            nc.sync.dma_start(out=bqkv_sb, in_=bqkv)
            ones_tok = singles.tile([1, TT], F16)
            nc.vector.memset(ones_tok, 1.0)
        if has_o_bias:
            bo_sb = singles.tile([128, 2], F32)
            nc.sync.dma_start(out=bo_sb, in_=bo.rearrange("(b p) o -> p (b o)", b=2))

        xc_r = xc.rearrange("(b p) n -> p b n", b=2)
        xt_r = xt.rearrange("(b p) n -> p b n", b=2)
        out_r = out.rearrange("(b p) n -> b p n", b=2)

        def emit_front(g):
            gtok = slice(g * GT, (g + 1) * GT)

            # ---- group load + fp16 casts (x-sum is folded into the matmuls).
            # Group 0 is primed per-tile so the DVE starts ~15us earlier.
            xc_t = xs_pool.tile([128, 2, GT], F32, tag="xc")
            xt_t = xs_pool.tile([128, 2, GT], F32, tag="xt")
            xc16 = x16_pool.tile([128, 2, GT], F16, tag="xc16")
            xt16 = x16_pool.tile([128, 2, GT], F16, tag="xt16")
            if g == 0:
                for j in range(G):
                    jt = slice(j * TT, (j + 1) * TT)
                    gjt = slice(g * GT + j * TT, g * GT + (j + 1) * TT)
                    nc.sync.dma_start(out=xc_t[:, :, jt], in_=xc_r[:, :, gjt])
                    nc.sync.dma_start(out=xt_t[:, :, jt], in_=xt_r[:, :, gjt])
                    nc.scalar.copy(out=xc16[:, :, jt], in_=xc_t[:, :, jt])
                    nc.scalar.copy(out=xt16[:, :, jt], in_=xt_t[:, :, jt])
            else:
                nc.sync.dma_start(out=xc_t, in_=xc_r[:, :, gtok])
                nc.sync.dma_start(out=xt_t, in_=xt_r[:, :, gtok])
                nc.scalar.copy(out=xc16, in_=xc_t)
                nc.scalar.copy(out=xt16, in_=xt_t)

            qkv = qkv_pool.tile([128, 3, G, C], F16, tag="qkv")
            gp = gp_pool.tile([128, G, NH, NH, HD], F16, tag="gp")

            # ---- per sub-tile qkv projection: q = Wq'(xc+xt) etc. via PSUM
            # accumulation over {xc,xt} x {channel-half b}; x-tile stationary.
            n_acc = 5 if has_qkv_bias else 4
            for j in range(G):
                qkv_ps = ps_qkv.tile([TT, 3 * C], F32, tag="qkvps")
                acc = 0
                for xsrc in (xc16, xt16):
                    for b in range(2):
                        lhs = xsrc[:, b, j * TT : (j + 1) * TT]
                        for lo, hi in ((0, 512), (512, 768)):
                            nc.tensor.matmul(
                                qkv_ps[:, lo:hi],
                                lhsT=lhs,
                                rhs=wqkv_sb[:, b, lo:hi],
                                start=(acc == 0),
                                stop=(acc == n_acc - 1),
                            )
                        acc += 1
                if has_qkv_bias:
                    for lo, hi in ((0, 512), (512, 768)):
                        nc.tensor.matmul(
                            qkv_ps[:, lo:hi],
                            lhsT=ones_tok,
                            rhs=bqkv_sb[:, lo:hi],
                            start=False,
                            stop=True,
                        )
                nc.scalar.copy(
                    out=qkv[:, :, j, :],
                    in_=qkv_ps.rearrange("p (r m) -> p r m", r=3),
                )

                # group-0 priming: per-tile gram as soon as each qkv lands
                if g == 0:
                    q_bj = (
                        qkv[:, 0, j]
                        .rearrange("p (h c) -> p h c", h=NH)
                        .unsqueeze(2)
                        .broadcast_to([TT, NH, NH, HD])
                    )
                    k_bj = (
                        qkv[:, 1, j]
                        .rearrange("p (g c) -> p g c", g=NH)
                        .unsqueeze(1)
                        .broadcast_to([TT, NH, NH, HD])
                    )
                    nc.vector.tensor_mul(gp[:, j], q_bj, k_bj)

            # ---- group gram multiply: ONE DVE op via 4-free-dim broadcast APs
            if g != 0:
                q_b = (
                    qkv[:, 0]
                    .rearrange("p t (h c) -> p t h c", h=NH)
                    .unsqueeze(3)
                    .broadcast_to([TT, G, NH, NH, HD])
                )
                k_b = (
                    qkv[:, 1]
                    .rearrange("p t (g c) -> p t g c", g=NH)
                    .unsqueeze(2)
                    .broadcast_to([TT, G, NH, NH, HD])
                )
                nc.vector.tensor_mul(gp, q_b, k_b)

            # ---- grouped gram c-tree (merged-stride 3D APs over (t,h,g))
            gpv = gp.rearrange("p t h g c -> p (t h g) c")  # [128, 256, 32]
            t16 = tree_pool.tile([128, G * 64, 16], F16, tag="t16")
            nc.vector.tensor_add(t16, gpv[:, :, 0:16], gpv[:, :, 16:32])
            t8 = tree2_pool.tile([128, G * 64, 8], F16, tag="t8")
            nc.vector.tensor_add(t8, t16[:, :, 0:8], t16[:, :, 8:16])
            t4 = tree2_pool.tile([128, G * 64, 4], F16, tag="t4")
            nc.vector.tensor_add(t4, t8[:, :, 0:4], t8[:, :, 4:8])
            t2 = tree2_pool.tile([128, G * 64, 2], F16, tag="t2")
            nc.vector.tensor_add(t2, t4[:, :, 0:2], t4[:, :, 2:4])
            dot4 = sm_pool.tile([128, G, NH, NH], F16, tag="dot4")
            nc.vector.tensor_add(
                dot4.rearrange("p t h g -> p (t h g)"), t2[:, :, 0], t2[:, :, 1]
            )

            # ---- grouped exp (ACT, softmax scale folded)
            ex4 = sm_pool.tile([128, G, NH, NH], F16, tag="ex4")
            nc.scalar.activation(
                out=ex4.rearrange("p t h g -> p (t h g)"),
                in_=dot4.rearrange("p t h g -> p (t h g)"),
                func=mybir.ActivationFunctionType.Exp,
                scale=SCALE,
            )
            return gtok, qkv, ex4

        def emit_tail(st):
            gtok, qkv, ex4 = st
            # ---- softmax tail (DVE); deferred one group so exp latency and
            # the next group's projection hide behind gram/tree work.
            den4 = sm_pool.tile([128, G * NH], F32, tag="den4")
            nc.vector.reduce_sum(
                den4, ex4.rearrange("p t h g -> p (t h) g"), axis=AX.X
            )
            rec4 = sm_pool.tile([128, G * NH], F16, tag="rec4")
            with nc.allow_low_precision(reason="softmax weights tolerate fp16"):
                nc.vector.reciprocal(rec4, den4)
            attn4 = sm_pool.tile([128, G, NH, NH], F16, tag="attn4")
            nc.vector.tensor_mul(
                attn4.rearrange("p t h g -> p (t h) g"),
                ex4.rearrange("p t h g -> p (t h) g"),
                rec4.unsqueeze(2).broadcast_to([128, G * NH, NH]),
            )

            # ---- group numerator multiply: ONE DVE op (attn x v broadcast).
            # pn [t, h, c, g8]; one DVE tree level folds g to GR=4, the
            # remaining g-sum happens on PE inside the g-expanded out-proj.
            pn = pn_pool.tile([128, G, NH, HD, NH], F16, tag="pn")
            a_b = attn4.unsqueeze(3).broadcast_to([TT, G, NH, HD, NH])
            v_b = (
                qkv[:, 2]
                .rearrange("p t (c g) -> p t c g", g=NH)
                .unsqueeze(2)
                .broadcast_to([TT, G, NH, HD, NH])
            )
            nc.vector.tensor_mul(pn, a_b, v_b)
            pn4 = pn_pool.tile([128, G, NH, HD, GR], F16, tag="pn4")
            pnv = pn.rearrange("p t h c g -> p (t h c) g")
            nc.vector.tensor_add(
                pn4.rearrange("p t h c g -> p (t h c) g"),
                pnv[:, :, 0:GR],
                pnv[:, :, GR : 2 * GR],
            )

            # ---- per sub-tile: KCH PE transposes pn4 -> channel-major PSUM,
            # one ACT evacuation per tile into the group otr buffer.
            otr_sb = otr_pool.tile([128, KCH, GT], F16, tag="otrsb")
            for j in range(G):
                pnf = pn4[:, j].rearrange("p h c g -> p (h c g)")  # [TT, 1024]
                tr_ps = ps_tr.tile([128, KCH, TT], F16, tag="trps")
                for ci in range(KCH):
                    nc.tensor.transpose(
                        tr_ps[:, ci], pnf[:, ci * 128 : (ci + 1) * 128], ident
                    )
                nc.scalar.copy(
                    out=otr_sb[:, :, j * TT : (j + 1) * TT], in_=tr_ps
                )

            # ---- out-projection with g-expanded Wo': contracts (h,c,g4)=1024
            # in KCH accumulating chunk-matmuls per 128-channel output half.
            out_ps = ps_out.tile([128, 2, GT], F32, tag="outps")
            for m in range(2):
                for ci in range(KCH):
                    nc.tensor.matmul(
                        out_ps[:, m],
                        lhsT=wox_sb[:, ci, m * 128 : (m + 1) * 128],
                        rhs=otr_sb[:, ci],
                        start=(ci == 0),
                        stop=(ci == KCH - 1),
                    )
            out_sb = out_pool.tile([128, 2, GT], F32, tag="outsb")
            if has_o_bias:
                for m in range(2):
                    nc.scalar.activation(
                        out=out_sb[:, m],
                        in_=out_ps[:, m],
                        func=mybir.ActivationFunctionType.Identity,
                        bias=bo_sb[:, m : m + 1],
                    )
            else:
                nc.scalar.copy(out=out_sb, in_=out_ps)
            for b in range(2):
                nc.sync.dma_start(out=out_r[b, :, gtok], in_=out_sb[:, b])

        prev = None
        for g in range(n_groups):
            st = emit_front(g)
            if prev is not None:
                emit_tail(prev)
            prev = st
        emit_tail(prev)

    nc.compile()
    return nc


# split_heads uses channel c*nh+h for (head h, dim c); permute projection rows
# so q,k come out head-major (h*32+c). v stays in natural order: its (c,g)
# interleave is exactly what the numerator multiply wants. The out-proj weight
# is expanded over g: pn layout is [h, c, g] so row (h*32+c)*8+g of Wo' equals
# WoT row h*32+c (Wo's input channels already match head-major merge order).
_PERM = np.array([c * NH + h for h in range(NH) for c in range(HD)])


def _prep_weights(Wq, bq, Wk, bk, Wv, bv, Wo, bo):
    wqkv = np.concatenate([Wq[_PERM].T, Wk[_PERM].T, Wv.T], axis=1)
    wqkv = np.ascontiguousarray(wqkv).astype(np.float16)
    wot = np.ascontiguousarray(Wo.T)  # [256 (h,c), 256 m]
    wox = np.repeat(wot, 4, axis=0).astype(np.float16)  # [(h,c,g4)=1024, 256]
    bqkv = np.concatenate([bq[_PERM], bk[_PERM], bv]).astype(np.float16)[None]
    bo_a = bo.astype(np.float32)[:, None]
    return wqkv, wox, bqkv, bo_a


def _in_maps(x_c, x_t, wqkv, wox, bqkv, bo_a):
    maps = []
    for b in range(B):
        maps.append(
            {
                "xc": np.ascontiguousarray(x_c[b].reshape(C, N)),
                "xt": np.ascontiguousarray(x_t[b].reshape(C, N)),
                "wqkv": wqkv,
                "wox": wox,
                "bqkv": bqkv,
                "bo": bo_a,
            }
        )
    return maps


def kernel(x_c, x_t, Wq, bq, Wk, bk, Wv, bv, Wo, bo):
    x_c = np.asarray(x_c, dtype=np.float32)
    x_t = np.asarray(x_t, dtype=np.float32)
    wqkv, wox, bqkv, bo_a = _prep_weights(
        np.asarray(Wq, np.float32),
        np.asarray(bq, np.float32),
        np.asarray(Wk, np.float32),
        np.asarray(bk, np.float32),
        np.asarray(Wv, np.float32),
        np.asarray(bv, np.float32),
        np.asarray(Wo, np.float32),
        np.asarray(bo, np.float32),
    )
    nc = build_kernel(
        has_qkv_bias=bool(np.any(bqkv)), has_o_bias=bool(np.any(bo_a))
    )
    res = run_bass_kernel_spmd(
        nc, _in_maps(x_c, x_t, wqkv, wox, bqkv, bo_a), list(range(B))
    )
    outs = [np.asarray(res.results[b]["out"]).reshape(C, HH, WW) for b in range(B)]
    return np.stack(outs).astype(np.float32)


def _install_ntff_shim():
    """Recreate the missing antenv.axon_hooks module + ctypes NTFF hook
    (mirrors trn_agent_boot.trn_boot's degraded-silently path). Test-only."""
    import contextlib
    import ctypes
    import types

    try:
        from antenv.axon_hooks import get_axon_ntff_profile_hook  # noqa: F401

        return True
    except ImportError:
        pass
    import antenv

    mod = types.ModuleType("antenv.axon_hooks")
    mod._hook = None

    def set_axon_ntff_profile_hook(h):
        mod._hook = h

    def get_axon_ntff_profile_hook():
        return mod._hook

    mod.set_axon_ntff_profile_hook = set_axon_ntff_profile_hook
    mod.get_axon_ntff_profile_hook = get_axon_ntff_profile_hook
    sys.modules["antenv.axon_hooks"] = mod
    antenv.axon_hooks = mod

    so_path = "/opt/axon/libaxon_pjrt.so"
    if not os.path.exists(so_path):
        return False
    lib = ctypes.CDLL(so_path)
    if not hasattr(lib, "axon_start_nrt_profile"):
        return False
    lib.axon_start_nrt_profile.argtypes = [
        ctypes.POINTER(ctypes.c_int64),
        ctypes.c_size_t,
    ]
    lib.axon_start_nrt_profile.restype = ctypes.c_int64
    lib.axon_stop_nrt_profile.argtypes = [ctypes.c_char_p]
    lib.axon_stop_nrt_profile.restype = ctypes.c_int64

    @contextlib.contextmanager
    def _hook(output_dir, device_ids):
        import jax

        jax.devices()
        if device_ids:
            ids = (ctypes.c_int64 * len(device_ids))(*device_ids)
            rc = lib.axon_start_nrt_profile(ids, len(device_ids))
        else:
            rc = lib.axon_start_nrt_profile(None, 0)
        if rc != 0:
            raise RuntimeError(f"axon_start_nrt_profile rc={rc}")
        try:
            yield
        finally:
            n = lib.axon_stop_nrt_profile(str(output_dir).encode())
            print(f"profile: {n} file(s) written to {output_dir}")

    set_axon_ntff_profile_hook(_hook)
    return True


def profile_run(inputs_np):
    """Run once more with NTFF tracing on core 0; return exec_time_ns."""
    import concourse.bass_utils as bu

    _install_ntff_shim()
    bu.upload_artifacts = lambda d: "local://" + d  # no S3 in this container
    x_c = np.asarray(inputs_np["x_c"], np.float32)
    x_t = np.asarray(inputs_np["x_t"], np.float32)
    wqkv, wox, bqkv, bo_a = _prep_weights(
        *[
            np.asarray(inputs_np[k], np.float32)
            for k in ("Wq", "bq", "Wk", "bk", "Wv", "bv", "Wo", "bo")
        ]
    )
    nc = build_kernel(
        has_qkv_bias=bool(np.any(bqkv)), has_o_bias=bool(np.any(bo_a))
    )
    res = run_bass_kernel_spmd(
        nc,
        _in_maps(x_c, x_t, wqkv, wox, bqkv, bo_a),
        list(range(B)),
        trace=True,
    )
    return res.exec_time_ns


if __name__ == "__main__":
    rng = np.random.default_rng(0)
    ins = {
        "x_c": rng.standard_normal((B, C, HH, WW), dtype=np.float32),
        "x_t": rng.standard_normal((B, C, HH, WW), dtype=np.float32),
        "Wq": (rng.standard_normal((C, C)) * 0.02).astype(np.float32),
        "bq": np.zeros(C, np.float32),
        "Wk": (rng.standard_normal((C, C)) * 0.02).astype(np.float32),
        "bk": np.zeros(C, np.float32),
        "Wv": (rng.standard_normal((C, C)) * 0.02).astype(np.float32),
        "bv": np.zeros(C, np.float32),
        "Wo": (rng.standard_normal((C, C)) * 0.02).astype(np.float32),
        "bo": np.zeros(C, np.float32),
    }
    out = kernel(**ins)
    print(out.shape, out.dtype)
